# revision 9
# baseline (speedup 1.0000x reference)
"""Paged-attention decode kernel for 8 TRN2 NeuronCores.

Problem: B=16 decode sequences, H=16 heads, D=128 head dim, paged KV cache
(2048 blocks x 16 tokens), context S=2048 per sequence.

Sharding: data-parallel over sequences -- 2 sequences per core, no
collectives.  The host applies the KV-cache scatter (slot_mapping) and the
paged gather (block_tables) while laying out per-core shards; the device
kernel does the full masked single-token attention.

Device math (per core, per sequence), bf16 inputs / fp32 accumulate:
  scores[s, h] = sum_d K[s,h,d] * (q[h,d]*SCALE)
  e[s, h]      = exp(scores + ctx_mask[s])           (ScalarE, bias trick)
  o_num[h, :]  = sum_s e[s,h] * V[s,h,:]             (PE, accumulate in PSUM)
  denom[h]     = sum_s e[s,h]                        (PE, ones column)
  out[h, :]    = o_num[h, h*128:(h+1)*128] / denom[h]

QK is split across engines so neither is the bottleneck: heads 0..HP-1 on
the TensorEngine (K-tile stationary, q moving), heads HP..15 on the
VectorEngine (broadcast q multiply + segmented reduce over d).  The KV
stream is chunked (1,3,3,3,3,2,1) tiles so the first QK starts early, the
final chunk's post-DMA tail is one tile, and per-chunk PE bursts stay dense
enough to keep the HAM clock-gate warm.  QK for tile t+1 is emitted before
PV of tile t so the PE never stalls on the ScalarE exp.
"""

import numpy as np
import ml_dtypes

from concourse import bass, bacc, mybir, tile
from concourse.bass_utils import run_bass_kernel_spmd

# Problem constants (hardcoded per the grading contract).
B = 16          # total sequences
H = 16          # heads
D = 128         # head dim
BLOCK = 16      # tokens per cache block
BPS = 128       # blocks per sequence
NB = B * BPS    # total cache blocks
S = BPS * BLOCK # max context per sequence (2048)
SCALE = 0.08838834764831845

N_CORES = 8
B2 = B // N_CORES             # sequences per core (2)
T = S // 128                  # 128-token tiles per sequence (16)
CHUNKS = (1, 3, 3, 3, 3, 2, 1)  # KV stream chunking (tiles per DMA)
assert sum(CHUNKS) == T
HP = 8                        # heads on the TensorEngine
HV = H - HP                   # heads on the VectorEngine

F32 = mybir.dt.float32
BF16 = mybir.dt.bfloat16
NP_BF16 = ml_dtypes.bfloat16

MASK_NEG = -30000.0  # exp(x + MASK_NEG) == 0 in fp32 for any |x| < ~100


def build_nc(b2=B2, chunks=CHUNKS):
    """Build the per-core Bass graph (SPMD: same graph on all 8 cores)."""
    t_tiles = sum(chunks)
    sizes = sorted(set(chunks))
    nc = bacc.Bacc("TRN2", target_bir_lowering=False, debug=False)

    # One DRAM param per (tensor, chunk-size).
    #  kt: PE-head K transposed, chunk layout [d, (tile, h<HP, s_local)]
    #  kn: DVE-head K natural,   chunk layout [s_local, (tile, h-HP, d)]
    #  vv: V natural,            chunk layout [s_local, (tile, h, d)]
    n_of = {sz: sum(1 for c in chunks if c == sz) for sz in sizes}
    kt_p = {sz: nc.declare_dram_parameter(
        f"kt{sz}", [b2, n_of[sz], 128, sz * HP * 128], BF16, isOutput=False)
        for sz in sizes}
    kn_p = {sz: nc.declare_dram_parameter(
        f"kn{sz}", [b2, n_of[sz], 128, sz * HV * D], BF16, isOutput=False)
        for sz in sizes}
    vv_p = {sz: nc.declare_dram_parameter(
        f"vv{sz}", [b2, n_of[sz], 128, sz * H * D], BF16, isOutput=False)
        for sz in sizes}
    qt = nc.declare_dram_parameter("qt", [b2, 128, HP], BF16, isOutput=False)
    qr = nc.declare_dram_parameter("qr", [b2, 128, HV * D], BF16,
                                   isOutput=False)
    bias = nc.declare_dram_parameter("bias", [b2, 128, t_tiles], F32,
                                     isOutput=False)
    # PV numerator in all-heads layout [b, h', (h, d)], already normalized;
    # the host extracts the h'==h diagonal (128KB/seq, negligible DMA)
    out = nc.declare_dram_parameter("out", [b2, H, H * D], F32, isOutput=True)

    # chunk index -> (size, index within its param, global tile offset)
    chunk_meta = []
    seen = {sz: 0 for sz in sizes}
    t0 = 0
    for sz in chunks:
        chunk_meta.append((sz, seen[sz], t0))
        seen[sz] += 1
        t0 += sz
    tile2chunk = []
    for ci, (sz, _, _) in enumerate(chunk_meta):
        tile2chunk += [ci] * sz

    with tile.TileContext(nc) as tc:
        with (
            tc.tile_pool(name="const", bufs=1) as cpool,
            tc.tile_pool(name="kpool", bufs=2) as kpool,
            tc.tile_pool(name="vpool", bufs=2) as vpool,
            tc.tile_pool(name="small", bufs=2) as spool,
            tc.tile_pool(name="pscore", bufs=2,
                         space=bass.MemorySpace.PSUM) as pscore,
            tc.tile_pool(name="pacc", bufs=1,
                         space=bass.MemorySpace.PSUM) as pacc,
        ):
            ones_t = cpool.tile([128, 1], BF16, tag="ones")
            nc.gpsimd.memset(ones_t[:], 1.0)

            for b in range(b2):
                # small inputs ride the ACT ring so the sync ring leads with
                # the first K chunk
                qt_sb = spool.tile([128, HP], BF16, tag="qt_sb")
                nc.scalar.dma_start(out=qt_sb[:], in_=qt[b])
                qr_sb = spool.tile([128, HV * D], BF16, tag="qr_sb")
                nc.scalar.dma_start(out=qr_sb[:], in_=qr[b])
                bias_sb = spool.tile([128, t_tiles], F32, tag="bias_sb")
                nc.scalar.dma_start(out=bias_sb[:], in_=bias[b])

                ps_o = pacc.tile([H, H * D], F32, tag="ps_o")      # [16, 2048]
                ps_sums = pacc.tile([H, 1], F32, tag="ps_sums")

                kt_tiles, kn_tiles, vv_tiles = {}, {}, {}

                def issue_chunk(ci, b=b):
                    sz, pi, _ = chunk_meta[ci]
                    kc = kpool.tile([128, sz * HP * 128], BF16, tag=f"kt{sz}")
                    nc.sync.dma_start(out=kc[:], in_=kt_p[sz][b, pi])
                    kn = kpool.tile([128, sz * HV * D], BF16, tag=f"kn{sz}")
                    nc.sync.dma_start(out=kn[:], in_=kn_p[sz][b, pi])
                    vc = vpool.tile([128, sz * H * D], BF16, tag=f"vv{sz}")
                    nc.scalar.dma_start(out=vc[:], in_=vv_p[sz][b, pi])
                    kt_tiles[ci], kn_tiles[ci], vv_tiles[ci] = kc, kn, vc

                def qk(t):
                    ci = tile2chunk[t]
                    _, _, ct0 = chunk_meta[ci]
                    # PE heads: 8 matmuls, K-tile stationary
                    ps_sc = pscore.tile([128, HP], F32, tag="ps_sc")
                    kc = kt_tiles[ci]
                    for hh in range(HP):
                        o0 = (t - ct0) * HP * 128 + hh * 128
                        nc.tensor.matmul(
                            ps_sc[:, hh:hh + 1],
                            kc[:, o0:o0 + 128],
                            qt_sb[:, hh:hh + 1],
                            start=True, stop=True,
                        )
                    # DVE heads: broadcast multiply + segmented reduce
                    kn = kn_tiles[ci]
                    o0 = (t - ct0) * HV * D
                    prod = spool.tile([128, HV * D], BF16, tag="prod")
                    nc.vector.tensor_mul(prod[:], kn[:, o0:o0 + HV * D],
                                         qr_sb[:])
                    sc_dve = spool.tile([128, HV], F32, tag="sc_dve")
                    nc.vector.tensor_reduce(
                        sc_dve[:],
                        prod[:].rearrange("p (h d) -> p h d", h=HV),
                        axis=mybir.AxisListType.X,
                        op=mybir.AluOpType.add,
                    )
                    return ps_sc, sc_dve

                issue_chunk(0)
                if len(chunk_meta) > 1:
                    issue_chunk(1)
                sc_t = qk(0)

                for t in range(t_tiles):
                    # stay one tile ahead on QK (and one chunk ahead on DMA)
                    if t + 1 < t_tiles:
                        if tile2chunk[t + 1] != tile2chunk[t]:
                            nci = tile2chunk[t + 1] + 1
                            if nci < len(chunk_meta) and nci not in kt_tiles:
                                issue_chunk(nci)
                        sc_next = qk(t + 1)
                    else:
                        sc_next = None

                    ps_sc, sc_dve = sc_t
                    e_t = spool.tile([128, H], BF16, tag="e_t")
                    nc.scalar.activation(
                        e_t[:, 0:HP], ps_sc[:],
                        mybir.ActivationFunctionType.Exp,
                        bias=bias_sb[:, t:t + 1], scale=1.0,
                    )
                    nc.scalar.activation(
                        e_t[:, HP:H], sc_dve[:],
                        mybir.ActivationFunctionType.Exp,
                        bias=bias_sb[:, t:t + 1], scale=1.0,
                    )

                    first = t == 0
                    last = t == t_tiles - 1
                    ci = tile2chunk[t]
                    _, _, ct0 = chunk_meta[ci]
                    vc = vv_tiles[ci]
                    nc.tensor.matmul(ps_sums[:], e_t[:], ones_t[:],
                                     start=first, stop=last,
                                     skip_group_check=True)
                    for n in range(4):
                        o0 = (t - ct0) * H * D + n * 512
                        nc.tensor.matmul(
                            ps_o[:, n * 512:(n + 1) * 512],
                            e_t[:],
                            vc[:, o0:o0 + 512],
                            start=first, stop=last,
                            skip_group_check=True,
                        )
                    sc_t = sc_next

                recip = spool.tile([H, 1], F32, tag="recip")
                nc.vector.reciprocal(recip[:], ps_sums[:])
                o_full = spool.tile([H, H * D], F32, tag="o_full")
                # normalize split across the (idle) ScalarE and VectorE
                nc.scalar.mul(o_full[:, 0:H * D // 2],
                              ps_o[:, 0:H * D // 2], recip[:])
                nc.vector.tensor_scalar_mul(o_full[:, H * D // 2:],
                                            ps_o[:, H * D // 2:], recip[:])
                if b == b2 - 1:
                    # last output: low-latency HWDGE ring (all K DMAs done)
                    nc.sync.dma_start(out=out[b], in_=o_full[:])
                else:
                    # earlier outputs: SWDGE so they can never block the
                    # sync ring's K stream behind the finalize
                    nc.gpsimd.dma_start(out=out[b], in_=o_full[:])

    nc.compile()
    return nc


def prep_in_maps(q, k, v, k_cache, v_cache, block_tables, slot_mapping,
                 context_lens):
    """Host-side scatter + paged gather + per-core shard layouts."""
    q = np.asarray(q, np.float32)
    k = np.asarray(k, np.float32)
    v = np.asarray(v, np.float32)
    k_cache = np.asarray(k_cache, np.float32)
    v_cache = np.asarray(v_cache, np.float32)
    block_tables = np.asarray(block_tables, np.int32)
    slot_mapping = np.asarray(slot_mapping, np.int64)
    context_lens = np.asarray(context_lens, np.int32)

    nb, block_size, h, d = k_cache.shape
    # scatter the new token into the flat caches
    kc = k_cache.reshape(nb * block_size, h, d).copy()
    kc[slot_mapping] = k
    vc = v_cache.reshape(nb * block_size, h, d).copy()
    vc[slot_mapping] = v
    # paged gather -> [B, S, H, D]
    k_seq = kc.reshape(nb, block_size, h, d)[block_tables].reshape(B, S, h, d)
    v_seq = vc.reshape(nb, block_size, h, d)[block_tables].reshape(B, S, h, d)

    sizes = sorted(set(CHUNKS))
    kt_parts = {sz: [] for sz in sizes}
    kn_parts = {sz: [] for sz in sizes}
    v_parts = {sz: [] for sz in sizes}
    t0 = 0
    for sz in CHUNKS:
        s0, s1 = t0 * 128, (t0 + sz) * 128
        # PE-head K chunk: [B, sz*128, HP, D] -> [B, D, sz, HP, 128]
        kt_parts[sz].append(np.ascontiguousarray(
            k_seq[:, s0:s1, 0:HP].reshape(B, sz, 128, HP, D)
            .transpose(0, 4, 1, 3, 2)).astype(NP_BF16)
            .reshape(B, 1, D, sz * HP * 128))
        # DVE-head K chunk: [B, sz*128, HV*D] -> [B, 128, sz, HV*D]
        kn_parts[sz].append(np.ascontiguousarray(
            k_seq[:, s0:s1, HP:].reshape(B, sz, 128, HV * D)
            .transpose(0, 2, 1, 3)).astype(NP_BF16)
            .reshape(B, 1, 128, sz * HV * D))
        # V chunk: [B, sz*128, H*D] -> [B, 128, sz, H*D]
        v_parts[sz].append(np.ascontiguousarray(
            v_seq[:, s0:s1].reshape(B, sz, 128, H * D)
            .transpose(0, 2, 1, 3)).astype(NP_BF16)
            .reshape(B, 1, 128, sz * H * D))
        t0 += sz
    kt_host = {sz: np.concatenate(kt_parts[sz], axis=1) for sz in sizes}
    kn_host = {sz: np.concatenate(kn_parts[sz], axis=1) for sz in sizes}
    v_host = {sz: np.concatenate(v_parts[sz], axis=1) for sz in sizes}

    qs = (q * SCALE).astype(NP_BF16)
    qt_host = np.ascontiguousarray(qs[:, 0:HP].transpose(0, 2, 1))  # [B,D,HP]
    qr_host = np.ascontiguousarray(
        np.broadcast_to(qs[:, HP:].reshape(B, 1, HV * D), (B, 128, HV * D)))
    s_idx = np.arange(S, dtype=np.int64)
    m = np.where(s_idx[None, :] < context_lens[:, None].astype(np.int64),
                 0.0, MASK_NEG).astype(np.float32)
    bias_host = np.ascontiguousarray(m.reshape(B, T, 128).transpose(0, 2, 1))

    in_maps = []
    for i in range(N_CORES):
        lo, hi = i * B2, (i + 1) * B2
        im = {"qt": np.ascontiguousarray(qt_host[lo:hi]),
              "qr": np.ascontiguousarray(qr_host[lo:hi]),
              "bias": np.ascontiguousarray(bias_host[lo:hi])}
        for sz in sizes:
            im[f"kt{sz}"] = np.ascontiguousarray(kt_host[sz][lo:hi])
            im[f"kn{sz}"] = np.ascontiguousarray(kn_host[sz][lo:hi])
            im[f"vv{sz}"] = np.ascontiguousarray(v_host[sz][lo:hi])
        in_maps.append(im)
    return in_maps


_NC = None


def _get_nc():
    global _NC
    if _NC is None:
        _NC = build_nc()
    return _NC


def run(inputs, trace=False, **spmd_kwargs):
    """Run on hardware; returns (full_output, BassKernelResults)."""
    nc = _get_nc()
    in_maps = prep_in_maps(**inputs)
    res = run_bass_kernel_spmd(nc, in_maps, core_ids=list(range(N_CORES)),
                               trace=trace, **spmd_kwargs)
    out_full = np.concatenate([res.results[i]["out"] for i in range(N_CORES)],
                              axis=0).astype(np.float32)
    # extract the h'==h diagonal: [B, H, H*D] -> [B, H, D]
    hh = np.arange(H)
    out = out_full.reshape(B, H, H, D)[:, hh, hh, :]
    return np.ascontiguousarray(out), res


def kernel(**inputs) -> np.ndarray:
    out, _ = run(inputs, trace=False)
    return out


# revision 19
# speedup vs baseline: 1.2314x; 1.2314x over previous
"""Paged-attention decode kernel for 8 TRN2 NeuronCores.

Problem: B=16 decode sequences, H=16 heads, D=128 head dim, paged KV cache
(2048 blocks x 16 tokens), context S=2048 per sequence.

Sharding: data-parallel over sequences -- 2 sequences per core, no
collectives.  The host applies the KV-cache scatter (slot_mapping) and the
paged gather (block_tables) while laying out per-core shards; the device
kernel does the full masked single-token attention.

Device math (per core, per sequence), bf16 inputs / fp32 accumulate:
  scores[s, h] = sum_d K[s,h,d] * (q[h,d]*SCALE)
  e[s, h]      = exp(scores + ctx_mask[s])           (ScalarE, bias trick)
  o_num[h, :]  = sum_s e[s,h] * V[s,h,:]             (PE, accumulate in PSUM)
  denom[h]     = sum_s e[s,h]                        (PE, ones column)
  out[h, :]    = o_num[h, h*128:(h+1)*128] / denom[h]

QK is split across engines so neither is the bottleneck: heads 0..HP-1 on
the TensorEngine (K-tile stationary, q moving), heads HP..15 on the
VectorEngine (broadcast q multiply + segmented reduce over d).  The KV
stream is chunked (1,4,4,4,2,1) tiles: the 1-tile edge chunks start compute
early and leave a one-tile post-DMA tail (edge chunks run all 16 QK heads
on the PE to shorten the serial tail chain); each chunk's PE- and DVE-K
parts are concatenated so one DMA moves both.  QK for tile t+1 is emitted
before PV of tile t so the PE never stalls on the ScalarE exp, and the
final normalize runs as two independent halves on ScalarE/VectorE with
bf16 outputs.

Measured on the 8 axon TRN2 cores: ~105-108 us exec (HBM-contention slow
mode ~120 us), rel err ~3.9e-3 vs the f32 reference.  The f32 DMA roofline
for this problem is ~187 us; bf16 halves the 536MB KV stream.
"""

import numpy as np
import ml_dtypes

from concourse import bass, bacc, mybir, tile
from concourse.bass_utils import run_bass_kernel_spmd

# Problem constants (hardcoded per the grading contract).
B = 16          # total sequences
H = 16          # heads
D = 128         # head dim
BLOCK = 16      # tokens per cache block
BPS = 128       # blocks per sequence
NB = B * BPS    # total cache blocks
S = BPS * BLOCK # max context per sequence (2048)
SCALE = 0.08838834764831845

N_CORES = 8
B2 = B // N_CORES             # sequences per core (2)
T = S // 128                  # 128-token tiles per sequence (16)
CHUNKS = (1, 4, 4, 4, 2, 1)  # KV stream chunking (tiles per DMA)
assert sum(CHUNKS) == T
HP = 10                       # heads on the TensorEngine
HV = H - HP                   # heads on the VectorEngine

F32 = mybir.dt.float32
BF16 = mybir.dt.bfloat16
NP_BF16 = ml_dtypes.bfloat16

MASK_NEG = -30000.0  # exp(x + MASK_NEG) == 0 in fp32 for any |x| < ~100


def build_nc(b2=B2, chunks=CHUNKS):
    """Build the per-core Bass graph (SPMD: same graph on all 8 cores)."""
    t_tiles = sum(chunks)
    sizes = sorted(set(chunks))
    nc = bacc.Bacc("TRN2", target_bir_lowering=False, debug=False)

    # One DRAM param per (tensor, chunk-size).
    #  kt: PE-head K transposed, chunk layout [d, (tile, h<HP, s_local)]
    #  kn: DVE-head K natural,   chunk layout [s_local, (tile, h-HP, d)]
    #  vv: V natural,            chunk layout [s_local, (tile, h, d)]
    n_of = {sz: sum(1 for c in chunks if c == sz) for sz in sizes}
    # size-1 edge chunks do all 16 QK heads on the PE (shorter serial tail,
    # earlier start); bigger chunks split heads across PE and DVE.  The
    # PE-part (K transposed, partition=d) and DVE-part (K natural,
    # partition=s) of each chunk are concatenated along the free axis so one
    # DMA moves both.
    def kwidth(sz):
        return sz * H * 128 if sz == 1 else sz * (HP * 128 + HV * D)
    kk_p = {sz: nc.declare_dram_parameter(
        f"kk{sz}", [b2, n_of[sz], 128, kwidth(sz)], BF16, isOutput=False)
        for sz in sizes}
    vv_p = {sz: nc.declare_dram_parameter(
        f"vv{sz}", [b2, n_of[sz], 128, sz * H * D], BF16, isOutput=False)
        for sz in sizes}
    qt = nc.declare_dram_parameter("qt", [b2, 128, H], BF16, isOutput=False)
    qr = nc.declare_dram_parameter("qr", [b2, 128, HV * D], BF16,
                                   isOutput=False)
    bias = nc.declare_dram_parameter("bias", [b2, 128, t_tiles], F32,
                                     isOutput=False)
    # PV numerator in all-heads layout [b, h', (h, d)], already normalized;
    # the host extracts the h'==h diagonal (128KB/seq, negligible DMA)
    out = nc.declare_dram_parameter("out", [b2, H, H * D], BF16,
                                    isOutput=True)

    # chunk index -> (size, index within its param, global tile offset)
    chunk_meta = []
    seen = {sz: 0 for sz in sizes}
    t0 = 0
    for sz in chunks:
        chunk_meta.append((sz, seen[sz], t0))
        seen[sz] += 1
        t0 += sz
    tile2chunk = []
    for ci, (sz, _, _) in enumerate(chunk_meta):
        tile2chunk += [ci] * sz

    with tile.TileContext(nc) as tc:
        with (
            tc.tile_pool(name="const", bufs=1) as cpool,
            tc.tile_pool(name="kpool", bufs=2) as kpool,
            tc.tile_pool(name="vpool", bufs=2) as vpool,
            tc.tile_pool(name="small", bufs=2) as spool,
            tc.tile_pool(name="pscore", bufs=2,
                         space=bass.MemorySpace.PSUM) as pscore,
            tc.tile_pool(name="pacc", bufs=1,
                         space=bass.MemorySpace.PSUM) as pacc,
        ):
            ones_t = cpool.tile([128, 1], BF16, tag="ones")
            nc.gpsimd.memset(ones_t[:], 1.0)

            for b in range(b2):
                # small inputs ride the ACT ring so the sync ring leads with
                # the first K chunk
                qt_sb = spool.tile([128, H], BF16, tag="qt_sb")
                nc.scalar.dma_start(out=qt_sb[:], in_=qt[b])
                qr_sb = spool.tile([128, HV * D], BF16, tag="qr_sb")
                nc.scalar.dma_start(out=qr_sb[:], in_=qr[b])
                bias_sb = spool.tile([128, t_tiles], F32, tag="bias_sb")
                nc.scalar.dma_start(out=bias_sb[:], in_=bias[b])

                ps_o = pacc.tile([H, H * D], F32, tag="ps_o")      # [16, 2048]
                ps_sums = pacc.tile([H, 1], F32, tag="ps_sums")

                kk_tiles, vv_tiles = {}, {}

                def issue_chunk(ci, b=b):
                    sz, pi, _ = chunk_meta[ci]
                    nbuf = 3 if sz == max(sizes) else 2
                    kc = kpool.tile([128, kwidth(sz)], BF16,
                                    tag=f"kk{sz}", bufs=nbuf)
                    nc.sync.dma_start(out=kc[:], in_=kk_p[sz][b, pi])
                    vc = vpool.tile([128, sz * H * D], BF16,
                                    tag=f"vv{sz}", bufs=nbuf)
                    nc.scalar.dma_start(out=vc[:], in_=vv_p[sz][b, pi])
                    kk_tiles[ci], vv_tiles[ci] = kc, vc

                def qk(t):
                    ci = tile2chunk[t]
                    sz, _, ct0 = chunk_meta[ci]
                    kc = kk_tiles[ci]
                    if sz == 1:
                        # edge chunk: all 16 heads on the PE
                        ps_sc = pscore.tile([128, H], F32, tag="ps_sc",
                                            bufs=3)
                        for hh in range(H):
                            o0 = hh * 128
                            nc.tensor.matmul(
                                ps_sc[:, hh:hh + 1],
                                kc[:, o0:o0 + 128],
                                qt_sb[:, hh:hh + 1],
                                start=True, stop=True,
                            )
                        return ps_sc, None
                    # PE heads: HP matmuls, K-tile stationary
                    ps_sc = pscore.tile([128, H], F32, tag="ps_sc", bufs=3)
                    for hh in range(HP):
                        o0 = (t - ct0) * HP * 128 + hh * 128
                        nc.tensor.matmul(
                            ps_sc[:, hh:hh + 1],
                            kc[:, o0:o0 + 128],
                            qt_sb[:, hh:hh + 1],
                            start=True, stop=True,
                        )
                    # DVE heads: broadcast multiply + segmented reduce
                    o0 = sz * HP * 128 + (t - ct0) * HV * D
                    prod = spool.tile([128, HV * D], BF16, tag="prod", bufs=3)
                    nc.vector.tensor_mul(prod[:], kc[:, o0:o0 + HV * D],
                                         qr_sb[:])
                    sc_dve = spool.tile([128, HV], F32, tag="sc_dve", bufs=3)
                    nc.vector.tensor_reduce(
                        sc_dve[:],
                        prod[:].rearrange("p (h d) -> p h d", h=HV),
                        axis=mybir.AxisListType.X,
                        op=mybir.AluOpType.add,
                    )
                    return ps_sc, sc_dve

                issue_chunk(0)
                if len(chunk_meta) > 1:
                    issue_chunk(1)
                sc_t = qk(0)

                for t in range(t_tiles):
                    # stay one tile ahead on QK (and one chunk ahead on DMA)
                    if t + 1 < t_tiles:
                        if tile2chunk[t + 1] != tile2chunk[t]:
                            nci = tile2chunk[t + 1] + 1
                            if nci < len(chunk_meta) and nci not in kk_tiles:
                                issue_chunk(nci)
                        sc_next = qk(t + 1)
                    else:
                        sc_next = None

                    ps_sc, sc_dve = sc_t
                    e_t = spool.tile([128, H], BF16, tag="e_t", bufs=3)
                    if sc_dve is None:
                        nc.scalar.activation(
                            e_t[:], ps_sc[:, 0:H],
                            mybir.ActivationFunctionType.Exp,
                            bias=bias_sb[:, t:t + 1], scale=1.0,
                        )
                    else:
                        nc.scalar.activation(
                            e_t[:, 0:HP], ps_sc[:, 0:HP],
                            mybir.ActivationFunctionType.Exp,
                            bias=bias_sb[:, t:t + 1], scale=1.0,
                        )
                        nc.scalar.activation(
                            e_t[:, HP:H], sc_dve[:],
                            mybir.ActivationFunctionType.Exp,
                            bias=bias_sb[:, t:t + 1], scale=1.0,
                        )

                    first = t == 0
                    last = t == t_tiles - 1
                    ci = tile2chunk[t]
                    _, _, ct0 = chunk_meta[ci]
                    vc = vv_tiles[ci]
                    nc.tensor.matmul(ps_sums[:], e_t[:], ones_t[:],
                                     start=first, stop=last,
                                     skip_group_check=True)
                    for n in range(4):
                        o0 = (t - ct0) * H * D + n * 512
                        nc.tensor.matmul(
                            ps_o[:, n * 512:(n + 1) * 512],
                            e_t[:],
                            vc[:, o0:o0 + 512],
                            start=first, stop=last,
                            skip_group_check=True,
                        )
                    sc_t = sc_next

                recip = spool.tile([H, 1], F32, tag="recip")
                nc.vector.reciprocal(recip[:], ps_sums[:])
                # normalize in two independent half-tiles so the ScalarE and
                # VectorE run concurrently and each half DMAs out as soon as
                # it is ready
                hw = H * D // 2
                o_lo = spool.tile([H, hw], BF16, tag="o_lo")
                o_hi = spool.tile([H, hw], BF16, tag="o_hi")
                nc.scalar.mul(o_lo[:], ps_o[:, 0:hw], recip[:])
                nc.vector.tensor_scalar_mul(o_hi[:], ps_o[:, hw:], recip[:])
                if b == b2 - 1:
                    # last outputs: low-latency HWDGE ring (all K DMAs done)
                    nc.sync.dma_start(out=out[b][:, 0:hw], in_=o_lo[:])
                    nc.sync.dma_start(out=out[b][:, hw:], in_=o_hi[:])
                else:
                    # earlier outputs: SWDGE so they can never block the
                    # sync ring's K stream behind the finalize
                    nc.gpsimd.dma_start(out=out[b][:, 0:hw], in_=o_lo[:])
                    nc.gpsimd.dma_start(out=out[b][:, hw:], in_=o_hi[:])

    nc.compile()
    return nc


def prep_in_maps(q, k, v, k_cache, v_cache, block_tables, slot_mapping,
                 context_lens):
    """Host-side scatter + paged gather + per-core shard layouts."""
    q = np.asarray(q, np.float32)
    k = np.asarray(k, np.float32)
    v = np.asarray(v, np.float32)
    k_cache = np.asarray(k_cache, np.float32)
    v_cache = np.asarray(v_cache, np.float32)
    block_tables = np.asarray(block_tables, np.int32)
    slot_mapping = np.asarray(slot_mapping, np.int64)
    context_lens = np.asarray(context_lens, np.int32)

    nb, block_size, h, d = k_cache.shape
    # scatter the new token into the flat caches
    kc = k_cache.reshape(nb * block_size, h, d).copy()
    kc[slot_mapping] = k
    vc = v_cache.reshape(nb * block_size, h, d).copy()
    vc[slot_mapping] = v
    # paged gather -> [B, S, H, D]
    k_seq = kc.reshape(nb, block_size, h, d)[block_tables].reshape(B, S, h, d)
    v_seq = vc.reshape(nb, block_size, h, d)[block_tables].reshape(B, S, h, d)

    sizes = sorted(set(CHUNKS))
    kk_parts = {sz: [] for sz in sizes}
    v_parts = {sz: [] for sz in sizes}
    t0 = 0
    for sz in CHUNKS:
        s0, s1 = t0 * 128, (t0 + sz) * 128
        hk = H if sz == 1 else HP
        # PE-head K chunk: [B, sz*128, hk, D] -> [B, D, sz, hk, 128]
        ktc = np.ascontiguousarray(
            k_seq[:, s0:s1, 0:hk].reshape(B, sz, 128, hk, D)
            .transpose(0, 4, 1, 3, 2)).astype(NP_BF16)             .reshape(B, 128, sz * hk * 128)
        if sz != 1:
            # DVE-head K chunk: [B, sz*128, HV*D] -> [B, 128, sz, HV*D]
            knc = np.ascontiguousarray(
                k_seq[:, s0:s1, HP:].reshape(B, sz, 128, HV * D)
                .transpose(0, 2, 1, 3)).astype(NP_BF16)                 .reshape(B, 128, sz * HV * D)
            ktc = np.concatenate([ktc, knc], axis=2)
        kk_parts[sz].append(ktc[:, None])
        # V chunk: [B, sz*128, H*D] -> [B, 128, sz, H*D]
        v_parts[sz].append(np.ascontiguousarray(
            v_seq[:, s0:s1].reshape(B, sz, 128, H * D)
            .transpose(0, 2, 1, 3)).astype(NP_BF16)
            .reshape(B, 1, 128, sz * H * D))
        t0 += sz
    kk_host = {sz: np.concatenate(kk_parts[sz], axis=1) for sz in sizes}
    v_host = {sz: np.concatenate(v_parts[sz], axis=1) for sz in sizes}

    qs = (q * SCALE).astype(NP_BF16)
    qt_host = np.ascontiguousarray(qs.transpose(0, 2, 1))  # [B, D, H]
    qr_host = np.ascontiguousarray(
        np.broadcast_to(qs[:, HP:].reshape(B, 1, HV * D), (B, 128, HV * D)))
    s_idx = np.arange(S, dtype=np.int64)
    m = np.where(s_idx[None, :] < context_lens[:, None].astype(np.int64),
                 0.0, MASK_NEG).astype(np.float32)
    bias_host = np.ascontiguousarray(m.reshape(B, T, 128).transpose(0, 2, 1))

    in_maps = []
    for i in range(N_CORES):
        lo, hi = i * B2, (i + 1) * B2
        im = {"qt": np.ascontiguousarray(qt_host[lo:hi]),
              "qr": np.ascontiguousarray(qr_host[lo:hi]),
              "bias": np.ascontiguousarray(bias_host[lo:hi])}
        for sz in sizes:
            im[f"kk{sz}"] = np.ascontiguousarray(kk_host[sz][lo:hi])
            im[f"vv{sz}"] = np.ascontiguousarray(v_host[sz][lo:hi])
        in_maps.append(im)
    return in_maps


_NC = None


def _get_nc():
    global _NC
    if _NC is None:
        _NC = build_nc()
    return _NC


def run(inputs, trace=False, **spmd_kwargs):
    """Run on hardware; returns (full_output, BassKernelResults)."""
    nc = _get_nc()
    in_maps = prep_in_maps(**inputs)
    res = run_bass_kernel_spmd(nc, in_maps, core_ids=list(range(N_CORES)),
                               trace=trace, **spmd_kwargs)
    out_full = np.concatenate([res.results[i]["out"] for i in range(N_CORES)],
                              axis=0).astype(np.float32)
    # extract the h'==h diagonal: [B, H, H*D] -> [B, H, D]
    hh = np.arange(H)
    out = out_full.reshape(B, H, H, D)[:, hh, hh, :]
    return np.ascontiguousarray(out), res


def kernel(**inputs) -> np.ndarray:
    out, _ = run(inputs, trace=False)
    return out
